# revision 24
# baseline (speedup 1.0000x reference)
"""Trainium2 Bass kernel for nn_GTO_Atten (token-compression attention).

Sharding: data-parallel. Core c owns batch b = c//2, node half h = c%2
(4096 of 8192 rows). Stage-1 token compression reduces over all 8192 nodes
of a batch element; each core computes partial (unnormalized) token sums +
partition functions (softmax without max-subtraction -- scores are ~1e-1),
then a pairwise AllReduce combines halves. Stage 2 is fully local.

All matmuls bf16 (psum fp32). Softmax normalizations ride the matmuls as
augmented ones-rows/columns; divisions happen on tiny tensors.

Layouts (per core, host-prepared):
  w0t   (512, 4096) bf16 : W0[b, half].T  (c on partitions)
  kv1w/q3w/projw (512, 512) bf16 : row-major weight
  qt    (128, 2048) bf16 : pair-packed Q^T * scale; rows 0:64 head 2p,
                           rows 64:128 head 2p+1, cols p*256:(p+1)*256
  qkv2k (128, 64)  bf16 : qkv2_w[:, :64] * scale, duplicated rows 0:64/64:128
  qkv2v (128, 64)  bf16 : qkv2_w[:, 64:], duplicated
  kv1b/q3b (128, 4) fp32 : bias.reshape(4,128).T  (col ci = bias[ci*128:+128])
Output: out (4096, 512) bf16 (host adds proj_b, casts fp32, concats).
"""

import numpy as np
import ml_dtypes

C = 512
H = 8
TD = 64
M = 256
S = 4096
CHUNK = 512
NCH = S // CHUNK   # 8
NTL = CHUNK // 128  # 4
NP = H // 2        # 4 head pairs

bf16 = ml_dtypes.bfloat16

_CACHE = {}


def _build_nc():
    from contextlib import ExitStack

    import concourse.bass as bass
    import concourse.mybir as mybir
    import concourse.tile as tile
    from concourse import bacc

    fp32 = mybir.dt.float32
    bf = mybir.dt.bfloat16
    AF = mybir.ActivationFunctionType
    RG = [[0, 1], [2, 3], [4, 5], [6, 7]]

    nc = bacc.Bacc()
    w0t = nc.declare_dram_parameter("w0t", [C, S], bf, isOutput=False)
    kv1w = nc.declare_dram_parameter("kv1w", [C, C], bf, isOutput=False)
    q3w = nc.declare_dram_parameter("q3w", [C, C], bf, isOutput=False)
    projw = nc.declare_dram_parameter("projw", [C, C], bf, isOutput=False)
    qt = nc.declare_dram_parameter("qt", [128, 2048], bf, isOutput=False)
    qkv2k = nc.declare_dram_parameter("qkv2k", [128, TD], bf, isOutput=False)
    qkv2v = nc.declare_dram_parameter("qkv2v", [128, TD], bf, isOutput=False)
    kv1b = nc.declare_dram_parameter("kv1b", [128, 4], fp32, isOutput=False)
    q3b = nc.declare_dram_parameter("q3b", [128, 4], fp32, isOutput=False)
    out = nc.declare_dram_parameter("out", [S, C], bf, isOutput=True)

    with ExitStack() as ctx:
        tc = ctx.enter_context(tile.TileContext(nc))
        const = ctx.enter_context(tc.tile_pool(name="const", bufs=1))
        w0p = ctx.enter_context(tc.tile_pool(name="w0p", bufs=2))
        k1tp = ctx.enter_context(tc.tile_pool(name="k1tp", bufs=2))
        augp = ctx.enter_context(tc.tile_pool(name="augp", bufs=2))
        e1p = ctx.enter_context(tc.tile_pool(name="e1p", bufs=3))
        q3tp = ctx.enter_context(tc.tile_pool(name="q3tp", bufs=1))
        smallp = ctx.enter_context(tc.tile_pool(name="smallp", bufs=2))
        e2p = ctx.enter_context(tc.tile_pool(name="e2p", bufs=8))
        wnp = ctx.enter_context(tc.tile_pool(name="wnp", bufs=3))
        wtp = ctx.enter_context(tc.tile_pool(name="wtp", bufs=6))
        zp = ctx.enter_context(tc.tile_pool(name="zp", bufs=6))
        outp = ctx.enter_context(tc.tile_pool(name="outp", bufs=4))
        usbp = ctx.enter_context(tc.tile_pool(name="usbp", bufs=8))

        psA = ExitStack()
        ps_s1 = psA.enter_context(tc.tile_pool(name="ps_s1", bufs=4, space="PSUM"))
        ps_pt = psA.enter_context(tc.tile_pool(name="ps_pt", bufs=2, space="PSUM"))
        ps_io = psA.enter_context(tc.tile_pool(name="ps_io", bufs=2, space="PSUM"))

        dram = ctx.enter_context(tc.tile_pool(name="dram", bufs=1, space="DRAM"))

        # ---- constants ----
        qt_s = const.tile([128, 2048], bf)
        nc.sync.dma_start(out=qt_s[:], in_=qt[:])
        k2k_s = const.tile([128, TD], bf)
        nc.sync.dma_start(out=k2k_s[:], in_=qkv2k[:])
        k2v_s = const.tile([128, TD], bf)
        nc.sync.dma_start(out=k2v_s[:], in_=qkv2v[:])
        kv1b_s = const.tile([128, 4], fp32)
        nc.sync.dma_start(out=kv1b_s[:], in_=kv1b[:])
        q3b_s = const.tile([128, 4], fp32)
        nc.sync.dma_start(out=q3b_s[:], in_=q3b[:])
        kv1w_s = []
        q3w_s = []
        projw_s = []
        for ci in range(4):
            t = const.tile([128, C], bf, tag=f"kv1w{ci}")
            nc.sync.dma_start(out=t[:], in_=kv1w[ci * 128:(ci + 1) * 128, :])
            kv1w_s.append(t)
            t = const.tile([128, C], bf, tag=f"q3w{ci}")
            nc.sync.dma_start(out=t[:], in_=q3w[ci * 128:(ci + 1) * 128, :])
            q3w_s.append(t)
            t = const.tile([128, C], bf, tag=f"projw{ci}")
            nc.sync.dma_start(out=t[:], in_=projw[ci * 128:(ci + 1) * 128, :])
            projw_s.append(t)

        # ---- PT accumulators (sbuf, fp32) ----
        pt_acc = [const.tile([TD + 1, M], fp32, tag=f"ptacc{h}", name=f"ptacc{h}")
                  for h in range(H)]
        for h in range(H):
            nc.any.memset(pt_acc[h][:], 0.0)

        # ---- stage 1 ----
        for ch in range(NCH):
            w0c = []
            for ci in range(4):
                t = w0p.tile([128, CHUNK], bf, tag=f"w0_{ci}")
                nc.sync.dma_start(
                    out=t[:],
                    in_=w0t[ci * 128:(ci + 1) * 128, ch * CHUNK:(ch + 1) * CHUNK],
                )
                w0c.append(t)
            # kv1^T chunk: (c, n) c on partitions
            kv1t_c = []
            for co in range(4):
                ps = ps_io.tile([128, CHUNK], fp32, tag="io")
                for ci in range(4):
                    nc.tensor.matmul(
                        ps[:],
                        kv1w_s[ci][:, co * 128:(co + 1) * 128],
                        w0c[ci][:],
                        start=(ci == 0),
                        stop=(ci == 3),
                    )
                t = k1tp.tile([128, CHUNK], bf, tag=f"k1t{co}")
                nc.scalar.activation(t[:], ps[:], AF.Identity, bias=kv1b_s[:, co:co + 1])
                kv1t_c.append(t)
            # kv1 natural (n, c) chunk -> aug tiles, no bias
            aug_c = []
            for ntl in range(NTL):
                ps = ps_io.tile([128, C], fp32, tag="io")
                for ci in range(4):
                    nc.tensor.matmul(
                        ps[:],
                        w0c[ci][:, ntl * 128:(ntl + 1) * 128],
                        kv1w_s[ci][:],
                        start=(ci == 0),
                        stop=(ci == 3),
                    )
                aug = augp.tile([128, H, TD + 1], bf, tag=f"aug{ntl}")
                nc.vector.tensor_copy(
                    aug[:, :, 0:TD],
                    ps[:].rearrange("p (h d) -> p h d", h=H),
                )
                nc.any.memset(aug[:, :, TD:TD + 1], 1.0)
                aug_c.append(aug)
            # S1' + exp + PT accumulation (per-chunk psum groups -> sbuf acc)
            for p in range(NP):
                e1_c = []
                for ntl in range(NTL):
                    s1e = ps_s1.tile([128, M], fp32, tag="s1")
                    nc.tensor.matmul(
                        s1e[:],
                        kv1t_c[p][0:64, ntl * 128:(ntl + 1) * 128],
                        qt_s[0:64, p * M:(p + 1) * M],
                        start=True, stop=True, tile_position=(0, 0),
                    )
                    s1o = ps_s1.tile([128, M], fp32, tag="s1")
                    nc.tensor.matmul(
                        s1o[:],
                        kv1t_c[p][64:128, ntl * 128:(ntl + 1) * 128],
                        qt_s[64:128, p * M:(p + 1) * M],
                        start=True, stop=True, tile_position=(64, 0),
                    )
                    e1 = e1p.tile([128, 2 * M], bf, tag=f"e1_{ntl}", name=f"e1_{ntl}")
                    nc.scalar.activation(e1[:, 0:M], s1e[:], AF.Exp)
                    nc.scalar.activation(e1[:, M:2 * M], s1o[:], AF.Exp)
                    e1_c.append(e1)
                for h in (2 * p, 2 * p + 1):
                    ptps = ps_pt.tile([TD + 1, M], fp32, tag="pt")
                    for ntl in range(NTL):
                        nc.tensor.matmul(
                            ptps[:],
                            aug_c[ntl][:, h, :],
                            e1_c[ntl][:, (h % 2) * M:(h % 2 + 1) * M],
                            start=(ntl == 0), stop=(ntl == NTL - 1),
                        )
                    nc.vector.tensor_add(pt_acc[h][:], pt_acc[h][:], ptps[:])

        # ---- collective combine of PT across the pair (2 pipelined halves) ----
        ccin_g = [dram.tile([4, TD + 1, M], fp32, tag=f"ccin{g}", name=f"ccin{g}")
                  for g in range(2)]
        ccout_g = [dram.tile([4, TD + 1, M], fp32, tag=f"ccout{g}", name=f"ccout{g}")
                   for g in range(2)]
        for g in range(2):
            for j in range(4):
                nc.sync.dma_start(out=ccin_g[g][j], in_=pt_acc[4 * g + j][:])
            nc.gpsimd.collective_compute(
                "AllReduce",
                mybir.AluOpType.add,
                replica_groups=RG,
                ins=[ccin_g[g].opt()],
                outs=[ccout_g[g].opt()],
            )

        # ---- q3^T (all chunks; fills PE while the collective runs) ----
        q3t_all = []
        for ch in range(NCH):
            w0c = []
            for ci in range(4):
                t = w0p.tile([128, CHUNK], bf, tag=f"w0_{ci}")
                nc.sync.dma_start(
                    out=t[:],
                    in_=w0t[ci * 128:(ci + 1) * 128, ch * CHUNK:(ch + 1) * CHUNK],
                )
                w0c.append(t)
            q3t_c = []
            for co in range(4):
                ps = ps_io.tile([128, CHUNK], fp32, tag="io")
                for ci in range(4):
                    nc.tensor.matmul(
                        ps[:],
                        q3w_s[ci][:, co * 128:(co + 1) * 128],
                        w0c[ci][:],
                        start=(ci == 0),
                        stop=(ci == 3),
                    )
                t = q3tp.tile([128, CHUNK], bf, tag=f"q3t_{ch}_{co}")
                nc.scalar.activation(t[:], ps[:], AF.Identity, bias=q3b_s[:, co:co + 1])
                q3t_c.append(t)
            q3t_all.append(q3t_c)

        psA.close()
        psB = ExitStack()
        ps_ktv = psB.enter_context(tc.tile_pool(name="ps_ktv", bufs=2, space="PSUM"))

        # ---- PT normalize + k^T, v (per collective half) ----
        ptu2 = const.tile([128, NP, M], fp32)
        zz = const.tile([128, 16], fp32)
        zd = dram.tile([1, 2048], fp32)
        z1b = const.tile([128, 2048], fp32)
        ptn = const.tile([128, NP, M], bf)
        for g in range(2):
            ccout = ccout_g[g]
            for pl in range(2):
                p = 2 * g + pl
                nc.sync.dma_start(out=ptu2[0:64, p, :], in_=ccout[2 * pl, 0:TD, :])
                nc.sync.dma_start(out=ptu2[64:128, p, :], in_=ccout[2 * pl + 1, 0:TD, :])
                for h2 in range(2):
                    nc.sync.dma_start(
                        out=zz[32 * p + 16 * h2:32 * p + 16 * h2 + 16, :],
                        in_=ccout[2 * pl + h2, TD:TD + 1, :].rearrange("o (a b) -> (o a) b", a=16),
                    )
            nc.vector.reciprocal(zz[64 * g:64 * g + 64, :], zz[64 * g:64 * g + 64, :])
            nc.sync.dma_start(out=zd[0, 1024 * g:1024 * g + 1024], in_=zz[64 * g:64 * g + 64, :])
            nc.sync.dma_start(
                out=z1b[:, 1024 * g:1024 * g + 1024],
                in_=zd[:, 1024 * g:1024 * g + 1024].broadcast_to((128, 1024)),
            )
        for p in range(NP):
            tmp_e = smallp.tile([64, M], fp32, tag="ptn_tmp")
            nc.vector.tensor_mul(
                tmp_e[:], ptu2[0:64, p, :], z1b[0:64, p * 512:p * 512 + M]
            )
            nc.scalar.activation(
                ptn[0:64, p, :], tmp_e[:], AF.Identity, bias=kv1b_s[0:64, p:p + 1]
            )
            tmp_o = smallp.tile([128, M], fp32, tag="ptn_tmp_o")
            nc.vector.tensor_mul(
                tmp_o[64:128, :],
                ptu2[64:128, p, :],
                z1b[64:128, p * 512 + M:p * 512 + 2 * M],
            )
            nc.scalar.activation(
                ptn[64:128, p, :], tmp_o[64:128, :], AF.Identity,
                bias=kv1b_s[64:128, p:p + 1],
            )

        kt_s = const.tile([128, NP, M], bf)
        vaug = [const.tile([128, 2, TD + 1], bf, tag=f"vaug{h}", name=f"vaug{h}") for h in range(H)]
        for p in range(NP):
            # k^T even head -> psum parts 0:64 -> direct copy
            kps_e = ps_ktv.tile([TD, M], fp32, tag="ktv")
            nc.tensor.matmul(
                kps_e[:], k2k_s[0:64, :], ptn[0:64, p, :],
                start=True, stop=True, tile_position=(0, 0),
            )
            nc.vector.tensor_copy(kt_s[0:64, p, :], kps_e[:])
            # k^T odd head -> psum parts 0:64 -> DMA shift to rows 64:128
            kps_o = ps_ktv.tile([TD, M], fp32, tag="ktv")
            nc.tensor.matmul(
                kps_o[:], k2k_s[64:128, :], ptn[64:128, p, :],
                start=True, stop=True, tile_position=(64, 0),
            )
            ktmp = smallp.tile([TD, M], bf, tag="ktmp")
            nc.vector.tensor_copy(ktmp[:], kps_o[:])
            nc.sync.dma_start(out=kt_s[64:128, p, :], in_=ktmp[:])
            # v: (m, td) full 128-partition outs
            for mc in range(2):
                vps_e = ps_ktv.tile([128, TD], fp32, tag="ktv")
                nc.tensor.matmul(
                    vps_e[:],
                    ptn[0:64, p, mc * 128:(mc + 1) * 128],
                    k2v_s[0:64, :],
                    start=True, stop=True, tile_position=(0, 0),
                )
                nc.vector.tensor_copy(vaug[2 * p][:, mc, 0:TD], vps_e[:])
                vps_o = ps_ktv.tile([128, TD], fp32, tag="ktv")
                nc.tensor.matmul(
                    vps_o[:],
                    ptn[64:128, p, mc * 128:(mc + 1) * 128],
                    k2v_s[64:128, :],
                    start=True, stop=True, tile_position=(64, 0),
                )
                nc.vector.tensor_copy(vaug[2 * p + 1][:, mc, 0:TD], vps_o[:])
        for h in range(H):
            nc.any.memset(vaug[h][:, :, TD:TD + 1], 1.0)

        psB.close()
        ps_u = ctx.enter_context(tc.tile_pool(name="ps_u", bufs=2, space="PSUM"))
        ps_s2 = ctx.enter_context(tc.tile_pool(name="ps_s2", bufs=2, space="PSUM"))
        ps_f = ctx.enter_context(tc.tile_pool(name="ps_f", bufs=2, space="PSUM"))

        # ---- stage 2 ----
        for ch in range(NCH):
            q3t_c = q3t_all[ch]
            wn_c = [wnp.tile([128, CHUNK], bf, tag=f"wn{p}", name=f"wn{p}") for p in range(NP)]
            for hh in range(2):  # half-chunk batches of 4 heads
                u_list = []
                zhc = zp.tile([128, 16], fp32, tag="zhc")
                for j in range(4):
                    h = 4 * hh + j
                    p, r = h // 2, (h % 2) * 64
                    ups = ps_u.tile([TD + 1, CHUNK], fp32, tag="u")
                    for mc in range(2):
                        s2 = ps_s2.tile([128, CHUNK], fp32, tag="s2")
                        nc.tensor.matmul(
                            s2[:],
                            kt_s[r:r + 64, p, mc * 128:(mc + 1) * 128],
                            q3t_c[p][r:r + 64, :],
                            start=True, stop=True, tile_position=(r, 0),
                        )
                        e2 = e2p.tile([128, CHUNK], bf, tag="e2")
                        nc.scalar.activation(e2[:], s2[:], AF.Exp)
                        nc.tensor.matmul(
                            ups[:], vaug[h][:, mc, :], e2[:],
                            start=(mc == 0), stop=(mc == 1),
                        )
                    usb = usbp.tile([TD, CHUNK], fp32, tag="usb")
                    nc.scalar.activation(usb[:], ups[0:TD, :], AF.Copy)
                    u_list.append(usb)
                    zs = zp.tile([TD + 1, CHUNK], fp32, tag="zs")
                    nc.vector.tensor_copy(zs[TD:TD + 1, :], ups[TD:TD + 1, :])
                    nc.sync.dma_start(
                        out=zhc[32 * j:32 * j + 32, :], in_=zs[TD:TD + 1, :]
                    )
                nc.vector.reciprocal(zhc[:], zhc[:])
                z2d = dram.tile([4, CHUNK], fp32, tag=f"z2d{hh}", name=f"z2d{hh}")
                for j in range(4):
                    nc.sync.dma_start(
                        out=z2d[j:j + 1, :], in_=zhc[32 * j:32 * j + 32, :]
                    )
                for j in range(4):
                    h = 4 * hh + j
                    p = h // 2
                    usb = u_list[j]
                    z2b = zp.tile([TD, CHUNK], fp32, tag="z2b")
                    nc.sync.dma_start(
                        out=z2b[:], in_=z2d[j:j + 1, :].broadcast_to((TD, CHUNK))
                    )
                    if h % 2 == 0:
                        nc.vector.tensor_mul(wn_c[p][0:64, :], usb[:], z2b[:])
                    else:
                        wtmp = wtp.tile([TD, CHUNK], bf, tag="wtmp")
                        nc.vector.tensor_mul(wtmp[:], usb[:], z2b[:])
                        nc.sync.dma_start(out=wn_c[p][64:128, :], in_=wtmp[:])
            for ntl in range(NTL):
                fe = ps_f.tile([128, C], fp32, tag="fe")
                fo = ps_f.tile([128, C], fp32, tag="fo")
                for p in range(NP):
                    nc.tensor.matmul(
                        fe[:],
                        wn_c[p][0:64, ntl * 128:(ntl + 1) * 128],
                        projw_s[p][0:64, :],
                        start=(p == 0), stop=(p == 3), tile_position=(0, 0),
                    )
                    nc.tensor.matmul(
                        fo[:],
                        wn_c[p][64:128, ntl * 128:(ntl + 1) * 128],
                        projw_s[p][64:128, :],
                        start=(p == 0), stop=(p == 3), tile_position=(64, 0),
                    )
                ot = outp.tile([128, C], bf, tag="ot")
                nc.vector.tensor_copy(ot[:], fe[:])
                nc.vector.tensor_add(ot[:], ot[:], fo[:])
                nc.sync.dma_start(
                    out=out[ch * CHUNK + ntl * 128: ch * CHUNK + (ntl + 1) * 128, :],
                    in_=ot[:],
                )

    nc.compile()
    return nc


def _get_nc():
    if "nc" not in _CACHE:
        _CACHE["nc"] = _build_nc()
    return _CACHE["nc"]


def make_in_maps(W0, Q, kv1_w, kv1_b, qkv2_w, q3_w, q3_b, proj_w, proj_b):
    scale = np.float32(1.0 / np.sqrt(TD))
    qt = np.zeros((128, 2048), dtype=bf16)
    for p in range(NP):
        qt[0:64, p * M:(p + 1) * M] = (Q[2 * p].T * scale).astype(bf16)
        qt[64:128, p * M:(p + 1) * M] = (Q[2 * p + 1].T * scale).astype(bf16)
    k2k = np.ascontiguousarray((qkv2_w[:, :TD] * scale)).astype(bf16)
    k2v = np.ascontiguousarray(qkv2_w[:, TD:]).astype(bf16)
    shared = {
        "kv1w": kv1_w.astype(bf16),
        "q3w": q3_w.astype(bf16),
        "projw": proj_w.astype(bf16),
        "qt": qt,
        "qkv2k": np.concatenate([k2k, k2k], axis=0),
        "qkv2v": np.concatenate([k2v, k2v], axis=0),
        "kv1b": np.ascontiguousarray(kv1_b.reshape(4, 128).T.astype(np.float32)),
        "q3b": np.ascontiguousarray(q3_b.reshape(4, 128).T.astype(np.float32)),
    }
    in_maps = []
    for core in range(8):
        b, half = core // 2, core % 2
        w0t = np.ascontiguousarray(W0[b, half * S:(half + 1) * S, :].T).astype(bf16)
        in_maps.append({**shared, "w0t": w0t})
    return in_maps


def _assemble(results, proj_b):
    outs = [np.asarray(results[i]["out"]).astype(np.float32) for i in range(8)]
    W = np.stack(
        [np.concatenate([outs[2 * b], outs[2 * b + 1]], axis=0) for b in range(4)],
        axis=0,
    )
    return (W + proj_b.astype(np.float32)).astype(np.float32)


def _install_profile_hook():
    """Provide antenv.axon_hooks (absent in this image) so that
    run_bass_kernel_spmd(trace=True) can capture NTFF profiles via the
    axon PJRT .so."""
    import sys
    import types
    import ctypes
    import contextlib

    if "antenv.axon_hooks" in sys.modules:
        return
    so_path = "/opt/axon/libaxon_pjrt.so"
    mod = types.ModuleType("antenv.axon_hooks")
    state = {"hook": None}

    def set_axon_ntff_profile_hook(h):
        state["hook"] = h

    def get_axon_ntff_profile_hook():
        return state["hook"]

    mod.set_axon_ntff_profile_hook = set_axon_ntff_profile_hook
    mod.get_axon_ntff_profile_hook = get_axon_ntff_profile_hook
    sys.modules["antenv.axon_hooks"] = mod

    try:
        lib = ctypes.CDLL(so_path)
    except OSError:
        return
    if not hasattr(lib, "axon_start_nrt_profile"):
        return
    lib.axon_start_nrt_profile.argtypes = [
        ctypes.POINTER(ctypes.c_int64), ctypes.c_size_t]
    lib.axon_start_nrt_profile.restype = ctypes.c_int64
    lib.axon_stop_nrt_profile.argtypes = [ctypes.c_char_p]
    lib.axon_stop_nrt_profile.restype = ctypes.c_int64

    @contextlib.contextmanager
    def _hook(output_dir, device_ids):
        import jax
        jax.devices()
        if device_ids:
            ids = (ctypes.c_int64 * len(device_ids))(*device_ids)
            rc = lib.axon_start_nrt_profile(ids, len(device_ids))
        else:
            rc = lib.axon_start_nrt_profile(None, 0)
        if rc != 0:
            raise RuntimeError(f"axon_start_nrt_profile rc={rc}")
        try:
            yield
        finally:
            n = lib.axon_stop_nrt_profile(str(output_dir).encode())
            print(f"profile: {n} file(s) written to {output_dir}")

    state["hook"] = _hook


def run(inputs, trace=False):
    from concourse.bass_utils import run_bass_kernel_spmd

    if trace:
        _install_profile_hook()

    nc = _get_nc()
    in_maps = make_in_maps(**inputs)
    res = run_bass_kernel_spmd(nc, in_maps, list(range(8)), trace=trace)
    out = _assemble(res.results, inputs["proj_b"])
    return out, res.exec_time_ns


def kernel(W0, Q, kv1_w, kv1_b, qkv2_w, q3_w, q3_b, proj_w, proj_b):
    inputs = dict(
        W0=np.asarray(W0, np.float32), Q=np.asarray(Q, np.float32),
        kv1_w=np.asarray(kv1_w, np.float32), kv1_b=np.asarray(kv1_b, np.float32),
        qkv2_w=np.asarray(qkv2_w, np.float32), q3_w=np.asarray(q3_w, np.float32),
        q3_b=np.asarray(q3_b, np.float32), proj_w=np.asarray(proj_w, np.float32),
        proj_b=np.asarray(proj_b, np.float32),
    )
    out, _ = run(inputs, trace=False)
    return out


# revision 25
# speedup vs baseline: 1.0289x; 1.0289x over previous
"""Trainium2 Bass kernel for nn_GTO_Atten (token-compression attention).

Sharding: data-parallel. Core c owns batch b = c//2, node half h = c%2
(4096 of 8192 rows). Stage-1 token compression reduces over all 8192 nodes
of a batch element; each core computes partial (unnormalized) token sums +
partition functions (softmax without max-subtraction -- scores are ~1e-1),
then a pairwise AllReduce combines halves. Stage 2 is fully local.

All matmuls bf16 (psum fp32). Softmax normalizations ride the matmuls as
augmented ones-rows/columns; divisions happen on tiny tensors.

Layouts (per core, host-prepared):
  w0t   (512, 4096) bf16 : W0[b, half].T  (c on partitions)
  kv1w/q3w/projw (512, 512) bf16 : row-major weight
  qt    (128, 2048) bf16 : pair-packed Q^T * scale; rows 0:64 head 2p,
                           rows 64:128 head 2p+1, cols p*256:(p+1)*256
  qkv2k (128, 64)  bf16 : qkv2_w[:, :64] * scale, duplicated rows 0:64/64:128
  qkv2v (128, 64)  bf16 : qkv2_w[:, 64:], duplicated
  kv1b/q3b (128, 4) fp32 : bias.reshape(4,128).T  (col ci = bias[ci*128:+128])
Output: out (4096, 512) bf16 (host adds proj_b, casts fp32, concats).
"""

import numpy as np
import ml_dtypes

C = 512
H = 8
TD = 64
M = 256
S = 4096
CHUNK = 512
NCH = S // CHUNK   # 8
NTL = CHUNK // 128  # 4
NP = H // 2        # 4 head pairs

bf16 = ml_dtypes.bfloat16

_CACHE = {}


def _build_nc():
    from contextlib import ExitStack

    import concourse.bass as bass
    import concourse.mybir as mybir
    import concourse.tile as tile
    from concourse import bacc

    fp32 = mybir.dt.float32
    bf = mybir.dt.bfloat16
    AF = mybir.ActivationFunctionType
    RG = [[0, 1], [2, 3], [4, 5], [6, 7]]

    nc = bacc.Bacc()
    w0t = nc.declare_dram_parameter("w0t", [C, S], bf, isOutput=False)
    kv1w = nc.declare_dram_parameter("kv1w", [C, C], bf, isOutput=False)
    q3w = nc.declare_dram_parameter("q3w", [C, C], bf, isOutput=False)
    projw = nc.declare_dram_parameter("projw", [C, C], bf, isOutput=False)
    qt = nc.declare_dram_parameter("qt", [128, 2048], bf, isOutput=False)
    qkv2k = nc.declare_dram_parameter("qkv2k", [128, TD], bf, isOutput=False)
    qkv2v = nc.declare_dram_parameter("qkv2v", [128, TD], bf, isOutput=False)
    kv1b = nc.declare_dram_parameter("kv1b", [128, 4], fp32, isOutput=False)
    q3b = nc.declare_dram_parameter("q3b", [128, 4], fp32, isOutput=False)
    out = nc.declare_dram_parameter("out", [S, C], bf, isOutput=True)

    with ExitStack() as ctx:
        tc = ctx.enter_context(tile.TileContext(nc))
        const = ctx.enter_context(tc.tile_pool(name="const", bufs=1))
        w0p = ctx.enter_context(tc.tile_pool(name="w0p", bufs=2))
        k1tp = ctx.enter_context(tc.tile_pool(name="k1tp", bufs=2))
        augp = ctx.enter_context(tc.tile_pool(name="augp", bufs=2))
        e1p = ctx.enter_context(tc.tile_pool(name="e1p", bufs=3))
        q3tp = ctx.enter_context(tc.tile_pool(name="q3tp", bufs=1))
        smallp = ctx.enter_context(tc.tile_pool(name="smallp", bufs=2))
        e2p = ctx.enter_context(tc.tile_pool(name="e2p", bufs=8))
        wnp = ctx.enter_context(tc.tile_pool(name="wnp", bufs=3))
        wtp = ctx.enter_context(tc.tile_pool(name="wtp", bufs=6))
        zp = ctx.enter_context(tc.tile_pool(name="zp", bufs=6))
        outp = ctx.enter_context(tc.tile_pool(name="outp", bufs=4))
        usbp = ctx.enter_context(tc.tile_pool(name="usbp", bufs=8))

        psA = ExitStack()
        ps_s1 = psA.enter_context(tc.tile_pool(name="ps_s1", bufs=4, space="PSUM"))
        ps_pt = psA.enter_context(tc.tile_pool(name="ps_pt", bufs=2, space="PSUM"))
        ps_io = psA.enter_context(tc.tile_pool(name="ps_io", bufs=2, space="PSUM"))

        dram = ctx.enter_context(tc.tile_pool(name="dram", bufs=1, space="DRAM"))

        # ---- constants ----
        qt_s = const.tile([128, 2048], bf)
        nc.sync.dma_start(out=qt_s[:], in_=qt[:])
        k2k_s = const.tile([128, TD], bf)
        nc.sync.dma_start(out=k2k_s[:], in_=qkv2k[:])
        k2v_s = const.tile([128, TD], bf)
        nc.sync.dma_start(out=k2v_s[:], in_=qkv2v[:])
        kv1b_s = const.tile([128, 4], fp32)
        nc.sync.dma_start(out=kv1b_s[:], in_=kv1b[:])
        q3b_s = const.tile([128, 4], fp32)
        nc.sync.dma_start(out=q3b_s[:], in_=q3b[:])
        kv1w_s = []
        q3w_s = []
        projw_s = []
        for ci in range(4):
            t = const.tile([128, C], bf, tag=f"kv1w{ci}")
            nc.sync.dma_start(out=t[:], in_=kv1w[ci * 128:(ci + 1) * 128, :])
            kv1w_s.append(t)
            t = const.tile([128, C], bf, tag=f"q3w{ci}")
            nc.sync.dma_start(out=t[:], in_=q3w[ci * 128:(ci + 1) * 128, :])
            q3w_s.append(t)
            t = const.tile([128, C], bf, tag=f"projw{ci}")
            nc.sync.dma_start(out=t[:], in_=projw[ci * 128:(ci + 1) * 128, :])
            projw_s.append(t)

        # ---- PT accumulators (sbuf, fp32) ----
        pt_acc = [const.tile([TD + 1, M], fp32, tag=f"ptacc{h}", name=f"ptacc{h}")
                  for h in range(H)]
        for h in range(H):
            nc.any.memset(pt_acc[h][:], 0.0)

        # ---- stage 1 ----
        for ch in range(NCH):
            w0c = []
            for ci in range(4):
                t = w0p.tile([128, CHUNK], bf, tag=f"w0_{ci}")
                nc.sync.dma_start(
                    out=t[:],
                    in_=w0t[ci * 128:(ci + 1) * 128, ch * CHUNK:(ch + 1) * CHUNK],
                )
                w0c.append(t)
            # kv1^T chunk: (c, n) c on partitions
            kv1t_c = []
            for co in range(4):
                ps = ps_io.tile([128, CHUNK], fp32, tag="io")
                for ci in range(4):
                    nc.tensor.matmul(
                        ps[:],
                        kv1w_s[ci][:, co * 128:(co + 1) * 128],
                        w0c[ci][:],
                        start=(ci == 0),
                        stop=(ci == 3),
                    )
                t = k1tp.tile([128, CHUNK], bf, tag=f"k1t{co}")
                nc.scalar.activation(t[:], ps[:], AF.Identity, bias=kv1b_s[:, co:co + 1])
                kv1t_c.append(t)
            # kv1 natural (n, c) chunk -> aug tiles, no bias
            aug_c = []
            for ntl in range(NTL):
                ps = ps_io.tile([128, C], fp32, tag="io")
                for ci in range(4):
                    nc.tensor.matmul(
                        ps[:],
                        w0c[ci][:, ntl * 128:(ntl + 1) * 128],
                        kv1w_s[ci][:],
                        start=(ci == 0),
                        stop=(ci == 3),
                    )
                aug = augp.tile([128, H, TD + 1], bf, tag=f"aug{ntl}")
                nc.vector.tensor_copy(
                    aug[:, :, 0:TD],
                    ps[:].rearrange("p (h d) -> p h d", h=H),
                )
                nc.any.memset(aug[:, :, TD:TD + 1], 1.0)
                aug_c.append(aug)
            # S1' + exp + PT accumulation (per-chunk psum groups -> sbuf acc)
            for p in range(NP):
                e1_c = []
                for ntl in range(NTL):
                    s1e = ps_s1.tile([128, M], fp32, tag="s1")
                    nc.tensor.matmul(
                        s1e[:],
                        kv1t_c[p][0:64, ntl * 128:(ntl + 1) * 128],
                        qt_s[0:64, p * M:(p + 1) * M],
                        start=True, stop=True, tile_position=(0, 0),
                    )
                    s1o = ps_s1.tile([128, M], fp32, tag="s1")
                    nc.tensor.matmul(
                        s1o[:],
                        kv1t_c[p][64:128, ntl * 128:(ntl + 1) * 128],
                        qt_s[64:128, p * M:(p + 1) * M],
                        start=True, stop=True, tile_position=(64, 0),
                    )
                    e1 = e1p.tile([128, 2 * M], bf, tag=f"e1_{ntl}", name=f"e1_{ntl}")
                    nc.scalar.activation(e1[:, 0:M], s1e[:], AF.Exp)
                    nc.scalar.activation(e1[:, M:2 * M], s1o[:], AF.Exp)
                    e1_c.append(e1)
                for h in (2 * p, 2 * p + 1):
                    ptps = ps_pt.tile([TD + 1, M], fp32, tag="pt")
                    for ntl in range(NTL):
                        nc.tensor.matmul(
                            ptps[:],
                            aug_c[ntl][:, h, :],
                            e1_c[ntl][:, (h % 2) * M:(h % 2 + 1) * M],
                            start=(ntl == 0), stop=(ntl == NTL - 1),
                        )
                    nc.vector.tensor_add(pt_acc[h][:], pt_acc[h][:], ptps[:])

        # ---- collective combine of PT across the pair ----
        ccin = dram.tile([H, TD + 1, M], fp32)
        ccout = dram.tile([H, TD + 1, M], fp32)
        for h in range(H):
            nc.sync.dma_start(out=ccin[h], in_=pt_acc[h][:])
        nc.gpsimd.collective_compute(
            "AllReduce",
            mybir.AluOpType.add,
            replica_groups=RG,
            ins=[ccin.opt()],
            outs=[ccout.opt()],
        )

        # ---- q3^T (all chunks; fills PE while the collective runs) ----
        q3t_all = []
        for ch in range(NCH):
            w0c = []
            for ci in range(4):
                t = w0p.tile([128, CHUNK], bf, tag=f"w0_{ci}")
                nc.sync.dma_start(
                    out=t[:],
                    in_=w0t[ci * 128:(ci + 1) * 128, ch * CHUNK:(ch + 1) * CHUNK],
                )
                w0c.append(t)
            q3t_c = []
            for co in range(4):
                ps = ps_io.tile([128, CHUNK], fp32, tag="io")
                for ci in range(4):
                    nc.tensor.matmul(
                        ps[:],
                        q3w_s[ci][:, co * 128:(co + 1) * 128],
                        w0c[ci][:],
                        start=(ci == 0),
                        stop=(ci == 3),
                    )
                t = q3tp.tile([128, CHUNK], bf, tag=f"q3t_{ch}_{co}")
                nc.scalar.activation(t[:], ps[:], AF.Identity, bias=q3b_s[:, co:co + 1])
                q3t_c.append(t)
            q3t_all.append(q3t_c)

        psA.close()
        psB = ExitStack()
        ps_ktv = psB.enter_context(tc.tile_pool(name="ps_ktv", bufs=2, space="PSUM"))

        # ---- PT normalize + k^T, v ----
        ptu2 = const.tile([128, NP, M], fp32)
        for p in range(NP):
            nc.sync.dma_start(out=ptu2[0:64, p, :], in_=ccout[2 * p, 0:TD, :])
            nc.sync.dma_start(out=ptu2[64:128, p, :], in_=ccout[2 * p + 1, 0:TD, :])
        zz = const.tile([128, 16], fp32)
        for p in range(NP):
            for h2 in range(2):
                nc.sync.dma_start(
                    out=zz[32 * p + 16 * h2:32 * p + 16 * h2 + 16, :],
                    in_=ccout[2 * p + h2, TD:TD + 1, :].rearrange("o (a b) -> (o a) b", a=16),
                )
        nc.vector.reciprocal(zz[:], zz[:])
        zd = dram.tile([1, 2048], fp32)
        nc.sync.dma_start(out=zd[0, :], in_=zz[:])
        z1b = const.tile([128, 2048], fp32)
        nc.sync.dma_start(out=z1b[:], in_=zd[:].broadcast_to((128, 2048)))
        ptn = const.tile([128, NP, M], bf)
        for p in range(NP):
            tmp_e = smallp.tile([64, M], fp32, tag="ptn_tmp")
            nc.vector.tensor_mul(
                tmp_e[:], ptu2[0:64, p, :], z1b[0:64, p * 512:p * 512 + M]
            )
            nc.scalar.activation(
                ptn[0:64, p, :], tmp_e[:], AF.Identity, bias=kv1b_s[0:64, p:p + 1]
            )
            tmp_o = smallp.tile([128, M], fp32, tag="ptn_tmp_o")
            nc.vector.tensor_mul(
                tmp_o[64:128, :],
                ptu2[64:128, p, :],
                z1b[64:128, p * 512 + M:p * 512 + 2 * M],
            )
            nc.scalar.activation(
                ptn[64:128, p, :], tmp_o[64:128, :], AF.Identity,
                bias=kv1b_s[64:128, p:p + 1],
            )

        kt_s = const.tile([128, NP, M], bf)
        vaug = [const.tile([128, 2, TD + 1], bf, tag=f"vaug{h}", name=f"vaug{h}") for h in range(H)]
        for p in range(NP):
            # k^T even head -> psum parts 0:64 -> direct copy
            kps_e = ps_ktv.tile([TD, M], fp32, tag="ktv")
            nc.tensor.matmul(
                kps_e[:], k2k_s[0:64, :], ptn[0:64, p, :],
                start=True, stop=True, tile_position=(0, 0),
            )
            nc.vector.tensor_copy(kt_s[0:64, p, :], kps_e[:])
            # k^T odd head -> psum parts 0:64 -> DMA shift to rows 64:128
            kps_o = ps_ktv.tile([TD, M], fp32, tag="ktv")
            nc.tensor.matmul(
                kps_o[:], k2k_s[64:128, :], ptn[64:128, p, :],
                start=True, stop=True, tile_position=(64, 0),
            )
            ktmp = smallp.tile([TD, M], bf, tag="ktmp")
            nc.vector.tensor_copy(ktmp[:], kps_o[:])
            nc.sync.dma_start(out=kt_s[64:128, p, :], in_=ktmp[:])
            # v: (m, td) full 128-partition outs
            for mc in range(2):
                vps_e = ps_ktv.tile([128, TD], fp32, tag="ktv")
                nc.tensor.matmul(
                    vps_e[:],
                    ptn[0:64, p, mc * 128:(mc + 1) * 128],
                    k2v_s[0:64, :],
                    start=True, stop=True, tile_position=(0, 0),
                )
                nc.vector.tensor_copy(vaug[2 * p][:, mc, 0:TD], vps_e[:])
                vps_o = ps_ktv.tile([128, TD], fp32, tag="ktv")
                nc.tensor.matmul(
                    vps_o[:],
                    ptn[64:128, p, mc * 128:(mc + 1) * 128],
                    k2v_s[64:128, :],
                    start=True, stop=True, tile_position=(64, 0),
                )
                nc.vector.tensor_copy(vaug[2 * p + 1][:, mc, 0:TD], vps_o[:])
        for h in range(H):
            nc.any.memset(vaug[h][:, :, TD:TD + 1], 1.0)

        psB.close()
        ps_u = ctx.enter_context(tc.tile_pool(name="ps_u", bufs=2, space="PSUM"))
        ps_s2 = ctx.enter_context(tc.tile_pool(name="ps_s2", bufs=2, space="PSUM"))
        ps_f = ctx.enter_context(tc.tile_pool(name="ps_f", bufs=2, space="PSUM"))

        # ---- stage 2 ----
        for ch in range(NCH):
            q3t_c = q3t_all[ch]
            wn_c = [wnp.tile([128, CHUNK], bf, tag=f"wn{p}", name=f"wn{p}") for p in range(NP)]
            for hh in range(2):  # half-chunk batches of 4 heads
                u_list = []
                zhc = zp.tile([128, 16], fp32, tag="zhc")
                for j in range(4):
                    h = 4 * hh + j
                    p, r = h // 2, (h % 2) * 64
                    ups = ps_u.tile([TD + 1, CHUNK], fp32, tag="u")
                    for mc in range(2):
                        s2 = ps_s2.tile([128, CHUNK], fp32, tag="s2")
                        nc.tensor.matmul(
                            s2[:],
                            kt_s[r:r + 64, p, mc * 128:(mc + 1) * 128],
                            q3t_c[p][r:r + 64, :],
                            start=True, stop=True, tile_position=(r, 0),
                        )
                        e2 = e2p.tile([128, CHUNK], bf, tag="e2")
                        nc.scalar.activation(e2[:], s2[:], AF.Exp)
                        nc.tensor.matmul(
                            ups[:], vaug[h][:, mc, :], e2[:],
                            start=(mc == 0), stop=(mc == 1),
                        )
                    usb = usbp.tile([TD, CHUNK], fp32, tag="usb")
                    nc.scalar.activation(usb[:], ups[0:TD, :], AF.Copy)
                    u_list.append(usb)
                    zs = zp.tile([TD + 1, CHUNK], fp32, tag="zs")
                    nc.vector.tensor_copy(zs[TD:TD + 1, :], ups[TD:TD + 1, :])
                    nc.sync.dma_start(
                        out=zhc[32 * j:32 * j + 32, :], in_=zs[TD:TD + 1, :]
                    )
                nc.vector.reciprocal(zhc[:], zhc[:])
                z2d = dram.tile([4, CHUNK], fp32, tag=f"z2d{hh}", name=f"z2d{hh}")
                for j in range(4):
                    nc.sync.dma_start(
                        out=z2d[j:j + 1, :], in_=zhc[32 * j:32 * j + 32, :]
                    )
                for j in range(4):
                    h = 4 * hh + j
                    p = h // 2
                    usb = u_list[j]
                    z2b = zp.tile([TD, CHUNK], fp32, tag="z2b")
                    nc.sync.dma_start(
                        out=z2b[:], in_=z2d[j:j + 1, :].broadcast_to((TD, CHUNK))
                    )
                    if h % 2 == 0:
                        nc.vector.tensor_mul(wn_c[p][0:64, :], usb[:], z2b[:])
                    else:
                        wtmp = wtp.tile([TD, CHUNK], bf, tag="wtmp")
                        nc.vector.tensor_mul(wtmp[:], usb[:], z2b[:])
                        nc.sync.dma_start(out=wn_c[p][64:128, :], in_=wtmp[:])
            for ntl in range(NTL):
                fe = ps_f.tile([128, C], fp32, tag="fe")
                fo = ps_f.tile([128, C], fp32, tag="fo")
                for p in range(NP):
                    nc.tensor.matmul(
                        fe[:],
                        wn_c[p][0:64, ntl * 128:(ntl + 1) * 128],
                        projw_s[p][0:64, :],
                        start=(p == 0), stop=(p == 3), tile_position=(0, 0),
                    )
                    nc.tensor.matmul(
                        fo[:],
                        wn_c[p][64:128, ntl * 128:(ntl + 1) * 128],
                        projw_s[p][64:128, :],
                        start=(p == 0), stop=(p == 3), tile_position=(64, 0),
                    )
                ot = outp.tile([128, C], bf, tag="ot")
                nc.vector.tensor_copy(ot[:], fe[:])
                nc.vector.tensor_add(ot[:], ot[:], fo[:])
                nc.sync.dma_start(
                    out=out[ch * CHUNK + ntl * 128: ch * CHUNK + (ntl + 1) * 128, :],
                    in_=ot[:],
                )

    nc.compile()
    return nc


def _get_nc():
    if "nc" not in _CACHE:
        _CACHE["nc"] = _build_nc()
    return _CACHE["nc"]


def make_in_maps(W0, Q, kv1_w, kv1_b, qkv2_w, q3_w, q3_b, proj_w, proj_b):
    scale = np.float32(1.0 / np.sqrt(TD))
    qt = np.zeros((128, 2048), dtype=bf16)
    for p in range(NP):
        qt[0:64, p * M:(p + 1) * M] = (Q[2 * p].T * scale).astype(bf16)
        qt[64:128, p * M:(p + 1) * M] = (Q[2 * p + 1].T * scale).astype(bf16)
    k2k = np.ascontiguousarray((qkv2_w[:, :TD] * scale)).astype(bf16)
    k2v = np.ascontiguousarray(qkv2_w[:, TD:]).astype(bf16)
    shared = {
        "kv1w": kv1_w.astype(bf16),
        "q3w": q3_w.astype(bf16),
        "projw": proj_w.astype(bf16),
        "qt": qt,
        "qkv2k": np.concatenate([k2k, k2k], axis=0),
        "qkv2v": np.concatenate([k2v, k2v], axis=0),
        "kv1b": np.ascontiguousarray(kv1_b.reshape(4, 128).T.astype(np.float32)),
        "q3b": np.ascontiguousarray(q3_b.reshape(4, 128).T.astype(np.float32)),
    }
    in_maps = []
    for core in range(8):
        b, half = core // 2, core % 2
        w0t = np.ascontiguousarray(W0[b, half * S:(half + 1) * S, :].T).astype(bf16)
        in_maps.append({**shared, "w0t": w0t})
    return in_maps


def _assemble(results, proj_b):
    outs = [np.asarray(results[i]["out"]).astype(np.float32) for i in range(8)]
    W = np.stack(
        [np.concatenate([outs[2 * b], outs[2 * b + 1]], axis=0) for b in range(4)],
        axis=0,
    )
    return (W + proj_b.astype(np.float32)).astype(np.float32)


def _install_profile_hook():
    """Provide antenv.axon_hooks (absent in this image) so that
    run_bass_kernel_spmd(trace=True) can capture NTFF profiles via the
    axon PJRT .so."""
    import sys
    import types
    import ctypes
    import contextlib

    if "antenv.axon_hooks" in sys.modules:
        return
    so_path = "/opt/axon/libaxon_pjrt.so"
    mod = types.ModuleType("antenv.axon_hooks")
    state = {"hook": None}

    def set_axon_ntff_profile_hook(h):
        state["hook"] = h

    def get_axon_ntff_profile_hook():
        return state["hook"]

    mod.set_axon_ntff_profile_hook = set_axon_ntff_profile_hook
    mod.get_axon_ntff_profile_hook = get_axon_ntff_profile_hook
    sys.modules["antenv.axon_hooks"] = mod

    try:
        lib = ctypes.CDLL(so_path)
    except OSError:
        return
    if not hasattr(lib, "axon_start_nrt_profile"):
        return
    lib.axon_start_nrt_profile.argtypes = [
        ctypes.POINTER(ctypes.c_int64), ctypes.c_size_t]
    lib.axon_start_nrt_profile.restype = ctypes.c_int64
    lib.axon_stop_nrt_profile.argtypes = [ctypes.c_char_p]
    lib.axon_stop_nrt_profile.restype = ctypes.c_int64

    @contextlib.contextmanager
    def _hook(output_dir, device_ids):
        import jax
        jax.devices()
        if device_ids:
            ids = (ctypes.c_int64 * len(device_ids))(*device_ids)
            rc = lib.axon_start_nrt_profile(ids, len(device_ids))
        else:
            rc = lib.axon_start_nrt_profile(None, 0)
        if rc != 0:
            raise RuntimeError(f"axon_start_nrt_profile rc={rc}")
        try:
            yield
        finally:
            n = lib.axon_stop_nrt_profile(str(output_dir).encode())
            print(f"profile: {n} file(s) written to {output_dir}")

    state["hook"] = _hook


def run(inputs, trace=False):
    from concourse.bass_utils import run_bass_kernel_spmd

    if trace:
        _install_profile_hook()

    nc = _get_nc()
    in_maps = make_in_maps(**inputs)
    res = run_bass_kernel_spmd(nc, in_maps, list(range(8)), trace=trace)
    out = _assemble(res.results, inputs["proj_b"])
    return out, res.exec_time_ns


def kernel(W0, Q, kv1_w, kv1_b, qkv2_w, q3_w, q3_b, proj_w, proj_b):
    inputs = dict(
        W0=np.asarray(W0, np.float32), Q=np.asarray(Q, np.float32),
        kv1_w=np.asarray(kv1_w, np.float32), kv1_b=np.asarray(kv1_b, np.float32),
        qkv2_w=np.asarray(qkv2_w, np.float32), q3_w=np.asarray(q3_w, np.float32),
        q3_b=np.asarray(q3_b, np.float32), proj_w=np.asarray(proj_w, np.float32),
        proj_b=np.asarray(proj_b, np.float32),
    )
    out, _ = run(inputs, trace=False)
    return out
